# revision 52
# baseline (speedup 1.0000x reference)
"""Gemma2 fused attention (B=1, S=4096, HID=2304, NH=8, NKV=4, HD=256,
sliding window 2048, softcap 50) on 8 Trainium2 NeuronCores.

Sharding: one query head per core (its GQA kv head recomputed per core);
o_proj is sharded over the contraction dim, per-core partials are summed
on the host.

Per-core math (core c, head h=c, kv group g=c//2):
  qT,kT = (W @ X.T) in [head_dim, tok] layout, RoPE'd on device (cos/sin
  tables precomputed on host; attention scale folded into Wq exactly).
  v in [tok, head_dim] layout.
  S.T[k,q] = kT.T @ qT; u = tanh(S.T/50); E = exp(50*u) in bf16
  (softcap bounds logits to +-50 so no max-subtraction is needed).
  Mask handled per 128(k) x 512(q) block: all-zero blocks skip the mask,
  fully-masked blocks are skipped entirely, causal/window boundary
  blocks zero E in place with an affine_select iota predicate (no mask
  data movement); irregular blocks (unreachable for causal+window masks)
  fall back to a host-packed additive mask stack.
  Z = ones.T @ E (PSUM row), attnT = (E @ v).T via lhsT=v chunks,
  Z accumulation runs on GpSimd to keep DVE free.
  out_partial[tok, 2304] = attnT.T @ WoT in bf16 with 1/Z fused into
  the PSUM->SBUF copy. Host sums the 8 partials in f32.
"""

import numpy as np
import ml_dtypes
from contextlib import ExitStack

import concourse.bass as bass
import concourse.tile as tile
import concourse.mybir as mybir
from concourse.bass_utils import run_bass_kernel_spmd
from concourse.vector_clock import ScopedClock

N_CORES = 8
HID = 2304
NH, NKV, HD = 8, 4, 256
SCALE = 256.0 ** -0.5
SOFTCAP = 50.0
ROPE_THETA = 10000.0
SLIDING_WINDOW = 2048
KC = HID // 128  # 18 contraction chunks for the projections

BF16 = mybir.dt.bfloat16
F32 = mybir.dt.float32
AF = mybir.ActivationFunctionType
ALU = mybir.AluOpType

TRACE = False  # test harness flips this to get NTFF exec time


class TC(tile.TileContext):
    """TileContext whose final drain splits sem waits one-per-instruction
    (this walrus rejects instructions carrying more than one wait)."""

    def _drain_and_barrier(self, tick_clock, wait_clock):
        probe = self.nc.sync.nop(nofuse=True, hint="drain_waits")
        wait_clock.add_sem_waits(
            probe.ins, ScopedClock({None: tick_clock.global_clock})
        )
        waits = list(probe.ins.sync_info.on_wait)
        probe.ins.sync_info.on_wait = waits[:1]
        rest = waits[1:]
        while rest:
            extra = self.nc.sync.nop(nofuse=True, hint="drain_waits")
            extra.ins.sync_info = mybir.SyncInfo(on_wait=rest[:1], on_update=[])
            rest = rest[1:]
        self.nc.sync.drain()
        self.nc.all_engine_barrier()
        popped = self.nc._tile_sem_poison_stack.pop()
        assert popped is self._sem_poison
        self.nc.clear_and_free_semaphores(list(self.sems.allocated().values()))
        self.nc.all_engine_barrier()


def split_multi_waits(nc):
    """Split multi-wait instructions: extras move onto same-engine NoOps
    inserted immediately before (engines execute in program order)."""
    ctr = 0
    for f in nc.m.functions:
        for b in f.blocks:
            insts = list(b.instructions)
            new = []
            changed = False
            for inst in insts:
                si = inst.sync_info
                if si is not None and len(si.on_wait) > 1:
                    waits = list(si.on_wait)
                    for w in waits[:-1]:
                        ctr += 1
                        nop = mybir.InstNoOp(
                            name=f"I-waitsplit-{ctr}",
                            engine=inst.engine,
                            debug=inst.debug,
                            sync_info=mybir.SyncInfo(on_wait=[w], on_update=[]),
                        )
                        new.append(nop)
                    inst.sync_info = mybir.SyncInfo(
                        on_wait=[waits[-1]], on_update=list(si.on_update)
                    )
                    changed = True
                new.append(inst)
            if changed:
                b.instructions = new
    return ctr


def _classify_mask(mask, S):
    """Per (k-chunk 128, q-block 512) block: 'skip' (fully masked),
    'clean' (all zero), 'causal'/'window' (affine boundary, handled with
    an iota select on device) or generic mixed (additive mask from a
    host-packed stack). Returns plan rows of (j, kind, arg)."""
    maskT = np.ascontiguousarray(np.asarray(mask, np.float32)[0, 0].T)  # [k, q]
    nj, nq = S // 128, S // 512
    blocks = maskT.reshape(nj, 128, nq, 512)
    mx = blocks.max(axis=(1, 3))
    mn = blocks.min(axis=(1, 3))
    skip = mx < -1e8
    clean = (mx == 0.0) & (mn == 0.0)
    kk = np.arange(128)[:, None]
    qq = np.arange(512)[None, :]
    plan = []
    mix_blocks = []

    def classify(j, qb):
        blk = maskT[j * 128:(j + 1) * 128, qb * 512:(qb + 1) * 512]
        allowed = blk == 0.0
        k_idx = j * 128 + kk
        q_idx = qb * 512 + qq
        if np.array_equal(allowed, k_idx <= q_idx):
            return ("causal", 0)
        if np.array_equal(allowed, k_idx > q_idx - SLIDING_WINDOW):
            return ("window", 0)
        mix_blocks.append((blk * (1.0 / SOFTCAP)).astype(np.float32))
        return ("mix", len(mix_blocks) - 1)

    for qb in range(nq):
        row = []
        for j in range(nj):
            if skip[j, qb]:
                continue
            if clean[j, qb]:
                row.append((j, "clean", 0, 0, 512))
            else:
                kind, arg = classify(j, qb)
                # narrow to one 256-wide q-half when the other is fully
                # masked (saves PE on the boundary chunks)
                blk = maskT[j * 128:(j + 1) * 128,
                            qb * 512:(qb + 1) * 512]
                allowed = blk == 0.0
                q0, qw = 0, 512
                if kind != "mix":
                    if not allowed[:, :256].any():
                        q0, qw = 256, 256
                    elif not allowed[:, 256:].any():
                        q0, qw = 0, 256
                row.append((j, kind, arg, q0, qw))
        if not row:
            # fully-masked q-block (unreachable for causal masks): keep the
            # diagonal chunks so the PSUM accumulations are still defined
            for j in range(4 * qb, 4 * qb + 4):
                mix_blocks.append(
                    (maskT[j * 128:(j + 1) * 128, qb * 512:(qb + 1) * 512]
                     * (1.0 / SOFTCAP)).astype(np.float32))
                row.append((j, "mix", len(mix_blocks) - 1, 0, 512))
        # the first chunk stays full width (initializes zacc, opens the
        # AV accumulation region); the group-closing chunk (n-3) too
        row[0] = row[0][:3] + (0, 512)
        if len(row) >= 4:
            row[-3] = row[-3][:3] + (0, 512)
        else:
            row = [r[:3] + (0, 512) for r in row]
        plan.append(row)
    if mix_blocks:
        maskb = np.stack(mix_blocks)
    else:
        maskb = None
    return plan, maskb


def _build(S, plan, nmix):
    """Emit the SPMD program (identical for all cores; only data differs)."""
    NT = S // 512  # token/query 512-blocks
    nc = bass.Bass("TRN2", target_bir_lowering=False, debug=False,
                   num_devices=N_CORES)

    xt_d = nc.dram_tensor("xt", [128, (S // 512) * KC * 512], BF16,
                          kind="ExternalInput")
    wqk_d = nc.dram_tensor("wqk", [128, KC * 512], BF16,
                           kind="ExternalInput")
    wo_d = nc.dram_tensor("wo", [128, 2 * HID], BF16, kind="ExternalInput")
    ident_d = nc.dram_tensor("ident", [128, 128], BF16,
                             kind="ExternalInput")
    cos_d = nc.dram_tensor("cost", [128, S], F32, kind="ExternalInput")
    sin_d = nc.dram_tensor("sint", [128, S], F32, kind="ExternalInput")
    if nmix:
        maskb_d = nc.dram_tensor("maskb", [nmix, 128, 512], F32,
                                 kind="ExternalInput")
    out_d = nc.dram_tensor("out", [128, (S // 128) * HID], BF16,
                           kind="ExternalOutput")
    # pair-exchange bounce buffers (collectives need internal DRAM)
    cc_in_d = nc.dram_tensor("cc_in", [NT, 128 * 1024], BF16)
    cc_out_d = nc.dram_tensor("cc_out", [NT, 2 * 128 * 1024], BF16)
    cc_warm_in_d = nc.dram_tensor("cc_warm_in", [256], BF16)
    cc_warm_out_d = nc.dram_tensor("cc_warm_out", [512], BF16)
    CC_GROUPS = [[2 * p, 2 * p + 1] for p in range(N_CORES // 2)]

    with ExitStack() as ctx:
        tc = ctx.enter_context(TC(nc))
        P = lambda name, bufs, space="SBUF": ctx.enter_context(
            tc.tile_pool(name=name, bufs=bufs, space=space))

        has_mix = any(r[1] == "mix" for row in plan for r in row)
        wpool = P("w", 1)
        xpool = P("x", 2)
        cspool = P("cs", 8)
        qpool = P("q", 8)
        qkpool = P("qk", 1)
        vpool = P("v", 1)
        tmppool = P("tmp", 3)
        upool = P("u", 3)
        epool = P("e", 7)
        mpool = P("m", 3) if has_mix else None
        apool = P("a", 4)
        zpool = P("z", 2)
        zapool = P("za", 2)
        opool = P("o", 2)
        rpool = P("r", 1)
        auxpool = P("aux", 2)
        kvpool = P("kv", 4)

        ps_qk = P("ps_qk", 2, "PSUM")
        ps_v = P("ps_v", 1, "PSUM")
        ps_s = P("ps_s", 3, "PSUM")
        ps_o = P("ps_o", 2, "PSUM")

        # --- resident weights / constants (wqk streams in chunk-
        # interleaved inside phase A block 0) ---
        wqk = wpool.tile([128, KC * 512], BF16, tag="wqk")
        wo = wpool.tile([128, 2 * HID], BF16, tag="wo")
        ident = wpool.tile([128, 128], BF16, tag="ident")
        nc.sync.dma_start(ident[:], ident_d[:, :])
        onesb = wpool.tile([128, 1], BF16, tag="onesb")
        nc.gpsimd.memset(onesb[:], 1.0)
        # boot the CC engine during phase A0 so the first real exchange
        # doesn't pay its ~25us startup cost
        nc.gpsimd.collective_compute(
            "AllGather", ALU.bypass, replica_groups=CC_GROUPS,
            ins=[cc_warm_in_d[:]], outs=[cc_warm_out_d[:]])

        # persistent activations (bf16, [128, S] each); q planes are
        # transient per-block tiles in qpool instead
        klo = qkpool.tile([128, S], BF16, tag="klo")
        khi = qkpool.tile([128, S], BF16, tag="khi")
        vt = vpool.tile([128, (S // 128) * 256], BF16, tag="vt")
        rc = rpool.tile([128, S // 128], F32, tag="rc")

        xt_tiles = {}
        cs_tiles = {}
        kv_tiles = {}
        q_tiles = {}
        a_tiles = {}

        def phase_a(T, skip_dma=False):
            """Return emission units (closures) for block T: q projection
            (rope'd locally) + 2 aux planes (pre-rope K halves on even
            cores, hd-major V halves on odd cores — the parity is entirely
            in the weight data), then the pair exchange of the aux planes.
            """
            c0 = T * 512
            if T in xt_tiles:
                xt = xt_tiles.pop(T)
            else:
                xt = xpool.tile([128, KC * 512], BF16, tag="xt")
            units = []

            def dma_unit():
                nc.sync.dma_start(xt[:], xt_d[:, T * (KC * 512):
                                              (T + 1) * (KC * 512)])
            if not skip_dma:
                units.append(dma_unit)

            cos = cspool.tile([128, 512], F32, tag="cos")
            sin = cspool.tile([128, 512], F32, tag="sin")
            cs_tiles[T] = (cos, sin)

            def cs_unit():
                nc.sync.dma_start(cos[:], cos_d[:, c0:c0 + 512])
                nc.sync.dma_start(sin[:], sin_d[:, c0:c0 + 512])
            units.append(cs_unit)

            pp = {}
            myaux = auxpool.tile([128, 1024], BF16, tag="myaux")
            qt_lo = qpool.tile([128, 512], BF16, tag="qt_lo")
            qt_hi = qpool.tile([128, 512], BF16, tag="qt_hi")
            q_tiles[T] = (qt_lo, qt_hi)

            def rope_q(plo, phi):
                t1 = tmppool.tile([128, 512], F32, tag="tmp")
                nc.vector.tensor_mul(t1[:], phi[:], sin[:])
                t2 = tmppool.tile([128, 512], F32, tag="tmp")
                nc.vector.tensor_mul(t2[:], plo[:], cos[:])
                nc.vector.tensor_sub(qt_lo[:], t2[:], t1[:])
                t3 = tmppool.tile([128, 512], F32, tag="tmp")
                nc.vector.tensor_mul(t3[:], plo[:], sin[:])
                t4 = tmppool.tile([128, 512], F32, tag="tmp")
                nc.vector.tensor_mul(t4[:], phi[:], cos[:])
                nc.vector.tensor_add(qt_hi[:], t4[:], t3[:])

            def qk_unit(ft):
                ps = ps_qk.tile([128, 512], F32, tag="ps_qk")
                for kc in range(KC):
                    nc.tensor.matmul(
                        ps[:],
                        wqk[:, kc * 512 + ft * 128: kc * 512 + ft * 128 + 128],
                        xt[:, kc * 512:(kc + 1) * 512],
                        start=(kc == 0), stop=(kc == KC - 1))
                pp[ft] = ps
                if ft == 1:
                    rope_q(pp[0], pp[1])
                elif ft >= 2:  # aux planes: plain copy to bf16
                    nc.vector.tensor_copy(
                        myaux[:, (ft - 2) * 512:(ft - 1) * 512], ps[:])

            def qk_chunk_major():
                # block 0 is paced by the weight/activation DMAs: keep 4
                # accumulations in flight (borrowing idle B-phase banks) so
                # each arriving chunk feeds 4 matmuls
                psA0 = ps_qk.tile([128, 512], F32, tag="ps_qk")
                psA1 = ps_qk.tile([128, 512], F32, tag="ps_qk")
                psA2 = ps_s.tile([128, 512], F32, tag="ps_s")
                psA3 = ps_o.tile([128, 512], F32, tag="ps_o")
                psA = [psA0, psA1, psA2, psA3]
                for kc in range(KC):
                    for ft in range(4):
                        nc.tensor.matmul(
                            psA[ft][:],
                            wqk[:, kc * 512 + ft * 128: kc * 512 + ft * 128 + 128],
                            xt[:, kc * 512:(kc + 1) * 512],
                            start=(kc == 0), stop=(kc == KC - 1))
                for h in range(2):
                    nc.vector.tensor_copy(
                        myaux[:, h * 512:(h + 1) * 512], psA[2 + h][:])
                rope_q(psA[0], psA[1])

            def x1_unit():
                # SBUF aux planes -> bounce -> pair AllGather
                nc.sync.dma_start(
                    cc_in_d[T, :].rearrange("(p f) -> p f", p=128),
                    myaux[:])
                nc.gpsimd.collective_compute(
                    "AllGather", ALU.bypass, replica_groups=CC_GROUPS,
                    ins=[cc_in_d[T, :]], outs=[cc_out_d[T, :]])

            def x2_unit():
                kv = kvpool.tile([128, 2048], BF16, tag="kv")
                kv_tiles[T] = kv
                # slot 0 = even core's planes (pre-rope K), slot 1 = odd
                # core's planes (hd-major V) — identical on both cores.
                nc.sync.dma_start(
                    kv[:, 0:1024],
                    cc_out_d[T, 0:128 * 1024].rearrange("(p f) -> p f",
                                                        p=128))
                nc.sync.dma_start(
                    kv[:, 1024:2048],
                    cc_out_d[T, 128 * 1024:].rearrange("(p f) -> p f",
                                                       p=128))

            # aux planes first so the pair exchange launches as early in
            # the previous B phase as possible; q (and its rope) follow
            if T == 0:
                units.append(qk_chunk_major)
                units.append(x1_unit)
                units.append(x2_unit)
            else:
                units.append(lambda: qk_unit(2))
                units.append(lambda: qk_unit(3))
                units.append(x1_unit)
                units.append(x2_unit)
                units.append(lambda: qk_unit(0))
                units.append(lambda: qk_unit(1))
            return units

        def rv_units(T):
            """Post-exchange: rope K (Vector) and transpose V (PE) for
            block T. Emitted at the start of phase B(T), by which time the
            exchange has long completed."""
            c0 = T * 512
            kv = kv_tiles.pop(T)
            cos, sin = cs_tiles.pop(T)
            kplo, kphi = kv[:, 0:512], kv[:, 512:1024]

            def rope_k():
                t1 = tmppool.tile([128, 512], F32, tag="tmp")
                nc.vector.tensor_mul(t1[:], kphi[:], sin[:])
                t2 = tmppool.tile([128, 512], F32, tag="tmp")
                nc.vector.tensor_mul(t2[:], kplo[:], cos[:])
                nc.vector.tensor_sub(klo[:, c0:c0 + 512], t2[:], t1[:])
                t3 = tmppool.tile([128, 512], F32, tag="tmp")
                nc.vector.tensor_mul(t3[:], kplo[:], sin[:])
                t4 = tmppool.tile([128, 512], F32, tag="tmp")
                nc.vector.tensor_mul(t4[:], kphi[:], cos[:])
                nc.vector.tensor_add(khi[:, c0:c0 + 512], t4[:], t3[:])

            def v_transpose(half):
                # [hd, tok] -> [tok, hd] via PE identity matmuls; one PSUM
                # tile packs (st, h2) x 2 = vt layout for 2 token chunks.
                ps = ps_v.tile([128, 512], F32, tag="ps_v")
                for i in range(2):
                    st = half * 2 + i
                    for h2 in range(2):
                        nc.tensor.matmul(
                            ps[:, i * 256 + h2 * 128:i * 256 + h2 * 128 + 128],
                            kv[:, 1024 + h2 * 512 + st * 128:
                               1024 + h2 * 512 + (st + 1) * 128],
                            ident[:], start=True, stop=True)
                tok = T * 4 + half * 2
                if half == 0:
                    nc.scalar.activation(vt[:, tok * 256:tok * 256 + 512],
                                         ps[:], AF.Copy)
                else:
                    nc.vector.tensor_copy(vt[:, tok * 256:tok * 256 + 512],
                                          ps[:])

            return [rope_k, lambda: v_transpose(0), lambda: v_transpose(1)]

        def a0_dma_unit():
            """Block-0 weight+activation DMA in 3-chunk slices so the
            first matmuls only wait for the first slice."""
            xt0 = xpool.tile([128, KC * 512], BF16, tag="xt")
            xt_tiles[0] = xt0
            for g in range(6):
                a, b = g * 1536, (g + 1) * 1536
                nc.sync.dma_start(wqk[:, a:b], wqk_d[:, a:b])
                nc.sync.dma_start(xt0[:, a:b], xt_d[:, a:b])

        def xt_prefetch_unit(T):
            """Issue block T's xt load early (queued behind block-0 DMAs)
            so the first woven qk units of phase A(T) never stall."""
            xt = xpool.tile([128, KC * 512], BF16, tag="xt")
            xt_tiles[T] = xt
            nc.sync.dma_start(xt[:], xt_d[:, T * (KC * 512):
                                          (T + 1) * (KC * 512)])

        def phase_b(qb):
            """Return emission units for attention q-block qb, one per
            k-chunk. The S matmuls of j lead the E-consumers of j-1 so the
            ACT chain has a full PE iteration of slack."""
            c0 = qb * 512
            zacc = zapool.tile([128, 512], F32, tag="za")
            olo = ps_o.tile([128, 512], F32, tag="ps_o")
            ohi = ps_o.tile([128, 512], F32, tag="ps_o")
            row = plan[qb]
            state = {}
            qt_lo, qt_hi = q_tiles.pop(qb)

            def s_unit(idx):
                j, kind, arg, q0, qw = row[idx]
                mk = None
                if kind == "mix":
                    mk = mpool.tile([128, 512], F32, tag="m")
                    nc.sync.dma_start(mk[:], maskb_d[arg, :, :])
                sps = ps_s.tile([128, 512], F32, tag="ps_s")
                nc.tensor.matmul(sps[:, :qw], klo[:, j * 128:(j + 1) * 128],
                                 qt_lo[:, q0:q0 + qw], start=True, stop=False)
                nc.tensor.matmul(sps[:, :qw], khi[:, j * 128:(j + 1) * 128],
                                 qt_hi[:, q0:q0 + qw], start=False, stop=True)
                e = epool.tile([128, 512], BF16, tag="e")
                u = upool.tile([128, 512], F32, tag="u")
                nc.scalar.activation(u[:, :qw], sps[:, :qw], AF.Tanh,
                                     scale=1.0 / SOFTCAP)
                if mk is not None:
                    u2 = upool.tile([128, 512], F32, tag="u")
                    nc.vector.tensor_add(u2[:, :qw], u[:, :qw], mk[:, :qw])
                    u = u2
                nc.scalar.activation(e[:, :qw], u[:, :qw], AF.Exp,
                                     scale=SOFTCAP)
                if kind == "causal":
                    # keep where q - k >= 0
                    nc.gpsimd.affine_select(
                        e[:, :qw], e[:, :qw], pattern=[[1, qw]],
                        compare_op=ALU.is_ge, fill=0.0,
                        base=qb * 512 + q0 - j * 128,
                        channel_multiplier=-1)
                elif kind == "window":
                    # keep where k - q + (SW - 1) >= 0
                    nc.gpsimd.affine_select(
                        e[:, :qw], e[:, :qw], pattern=[[-1, qw]],
                        compare_op=ALU.is_ge, fill=0.0,
                        base=j * 128 - (qb * 512 + q0) + SLIDING_WINDOW - 1,
                        channel_multiplier=1)
                if idx == 0:
                    nc.vector.tensor_copy(zacc[:], e[:])
                else:
                    nc.vector.tensor_add(zacc[:, q0:q0 + qw],
                                         zacc[:, q0:q0 + qw], e[:, :qw])
                state[idx] = e

            def mm_unit(idx, first, last):
                j, _, _, q0, qw = row[idx]
                e = state.pop(idx)
                nc.tensor.matmul(olo[:, q0:q0 + qw],
                                 vt[:, j * 256:j * 256 + 128], e[:, :qw],
                                 start=first, stop=last)
                nc.tensor.matmul(ohi[:, q0:q0 + qw],
                                 vt[:, j * 256 + 128:(j + 1) * 256],
                                 e[:, :qw], start=first, stop=last)

            def tail_unit():
                atlo = apool.tile([128, 512], BF16, tag="atlo")
                athi = apool.tile([128, 512], BF16, tag="athi")
                a_tiles[qb] = (atlo, athi)
                nc.vector.tensor_copy(atlo[:], olo[:])
                nc.vector.tensor_copy(athi[:], ohi[:])
                # Z transposed into [tok, 1] lanes: lhsT=zacc slice (bf16)
                # x ones -> PSUM [128, 4]; reciprocal straight from PSUM.
                zb = zpool.tile([128, 512], BF16, tag="z")
                nc.vector.tensor_copy(zb[:], zacc[:])
                zps = ps_s.tile([128, 4], F32, tag="ps_s")
                for sl in range(4):
                    nc.tensor.matmul(zps[:, sl:sl + 1],
                                     zb[:, sl * 128:(sl + 1) * 128],
                                     onesb[:], start=True, stop=True)
                nc.vector.reciprocal(rc[:, 4 * qb:4 * qb + 4], zps[:])

            # AV accumulation order: chunk 0 (full width) opens the PSUM
            # region, the always-full d1 chunk (n-3) closes it, so the
            # narrowed boundary chunks sit in the interior of the group.
            n = len(row)
            if n < 4:
                # short rows (not produced by causal+window masks): plain
                # in-order accumulation, no narrowing (widths forced 512)
                units = [lambda: s_unit(0)]
                for idx in range(1, n):
                    units.append(lambda idx=idx: (
                        s_unit(idx), mm_unit(idx - 1, idx - 1 == 0, False)))
                units.append(lambda: (mm_unit(n - 1, n == 1, True),
                                      tail_unit()))
                return units
            units = [lambda: s_unit(0)]
            for idx in range(1, n):
                if idx - 1 <= n - 4:
                    units.append(lambda idx=idx: (
                        s_unit(idx),
                        mm_unit(idx - 1, idx - 1 == 0, False)))
                else:
                    units.append(lambda idx=idx: s_unit(idx))
            units.append(lambda: mm_unit(n - 2, False, False))
            units.append(lambda: (mm_unit(n - 1, False, False),
                                  mm_unit(n - 3, False, True),
                                  tail_unit()))
            return units

        # PE warmup: a few throwaway matmuls so HAM reaches 8/8 before
        # the first real accumulation
        scratch = wpool.tile([128, 512], BF16, tag="scratch")
        nc.gpsimd.memset(scratch[:], 0.0)
        wps = ps_s.tile([128, 512], F32, tag="ps_s")
        for _ in range(12):
            nc.tensor.matmul(wps[:], scratch[:, :128], scratch[:],
                             start=True, stop=True)

        # output projection units (one per (tok-tile, feat-block)); the
        # 1/Z normalization is fused into the PSUM->SBUF copy. These are
        # woven into later B phases so the output DMA spreads over the
        # whole kernel instead of saturating the tail.
        fbs = [(0, 512), (512, 512), (1024, 512), (1536, 512), (2048, 256)]

        osb_tiles = {}

        def proj_unit(t, fi):
            f0, fw = fbs[fi]
            pool = ps_qk if fi % 3 < 2 else ps_v
            ps = pool.tile([128, 512], F32, tag=pool.name)
            atlo, athi = a_tiles[t // 4]
            sl = (t % 4) * 128
            nc.tensor.matmul(ps[:, :fw], atlo[:, sl:sl + 128],
                             wo[:, f0:f0 + fw], start=True, stop=False)
            nc.tensor.matmul(ps[:, :fw], athi[:, sl:sl + 128],
                             wo[:, HID + f0:HID + f0 + fw],
                             start=False, stop=True)
            if fi == 0:
                osb = opool.tile([128, HID], BF16, tag="o", name="osb")
                osb_tiles[t] = osb
            else:
                osb = osb_tiles[t]
            if fi % 2 == 0:
                nc.scalar.activation(osb[:, f0:f0 + fw], ps[:, :fw], AF.Copy,
                                     scale=rc[:, t:t + 1])
            else:
                nc.vector.tensor_scalar_mul(osb[:, f0:f0 + fw], ps[:, :fw],
                                            rc[:, t:t + 1])
            if fi == len(fbs) - 1:
                nc.gpsimd.dma_start(out_d[:, t * HID:(t + 1) * HID],
                                    osb_tiles.pop(t)[:])

        def phase_c(qb):
            return [lambda t=t, fi=fi: proj_unit(t, fi)
                    for t in range(4 * qb, 4 * qb + 4)
                    for fi in range(len(fbs))]

        def interleave_aunits(apart, cpart, n_dma):
            """Order the A(T+1)/C(T-1) filler: DMA/cos-sin issue units
            first (they start the loads), then proj units (always ready)
            to cover the xt DMA latency, then alternate qk/v with the
            remaining proj units."""
            if not cpart:
                return apart
            prefix, compute = apart[:n_dma], apart[n_dma:]
            lead, rest = cpart[:6], cpart[6:]
            out = list(prefix) + list(lead)
            ai, ri = 0, 0
            # alternate: one a-unit, one c-unit until both exhausted
            while ai < len(compute) or ri < len(rest):
                if ai < len(compute):
                    out.append(compute[ai]); ai += 1
                if ri < len(rest):
                    out.append(rest[ri]); ri += 1
            return out

        def weave(bunits, aunits):
            """Alternate B and A units so stalled B consumers never block
            independent A matmuls in the in-order PE queue."""
            out = []
            na, nb = len(aunits), len(bunits)
            ai = 0
            for bi, bu in enumerate(bunits):
                out.append(bu)
                want = (bi + 1) * na // nb
                while ai < want:
                    out.append(aunits[ai])
                    ai += 1
            out.extend(aunits[ai:])
            return out

        a0_dma_unit()
        a0_units = phase_a(0, skip_dma=True)  # [cs, qk_chunk_major, x1, x2]
        a0_units[0]()   # cos/sin block 0
        xt_prefetch_unit(1)
        nc.sync.dma_start(wo[:], wo_d[:, :])  # not needed until C(0) in B1
        for u in a0_units[1:]:
            u()
        # 3-phase lookahead: A0..A2 are emitted fully before B0 (A2's
        # matmuls cover the CC-engine boot window), then A(T+3) weaves
        # into B(T) -- every pair exchange gets ~2 phases of slack
        xt_prefetch_unit(2)
        for u in phase_a(1, skip_dma=True):
            u()
        for u in phase_a(2, skip_dma=True):
            u()
        for T in range(NT):
            # rv (rope K / transpose V of block T) sits near the phase
            # start: its exchange completed phases ago
            bu = phase_b(T)
            ins_at = 0 if T == 0 else min(2, len(bu))
            bunits = bu[:ins_at] + rv_units(T) + bu[ins_at:]
            aunits = phase_a(T + 3) if T + 3 < NT else []
            cunits = phase_c(T - 1) if T >= 1 else []
            with nc.named_scope(f"B{T}"):
                for u in weave(bunits,
                               interleave_aunits(aunits, cunits, 2)):
                    u()
        with nc.named_scope("Ctail"):
            for qb in (NT - 1,):
                for u in phase_c(qb):
                    u()

    split_multi_waits(nc)
    return nc


def _pack_rows(a):
    """[KC*128, C] -> [128, KC*C]: the SBUF-resident layout, so device
    loads are single contiguous DMAs with multi-KB partition lines."""
    r, c = a.shape
    return np.ascontiguousarray(
        a.reshape(r // 128, 128, c).transpose(1, 0, 2).reshape(128, -1))


def kernel(hidden_states, attention_mask, position_ids, Wqkv, Wo):
    bf16 = ml_dtypes.bfloat16
    hidden = np.asarray(hidden_states, np.float32)
    S = hidden.shape[1]
    X = hidden[0]  # [S, HID]
    XT = np.ascontiguousarray(X.T).astype(bf16)  # [HID, S]
    # xt_p[p, T*(KC*512) + kc*512 + s] = XT[kc*128+p, T*512+s]
    xt_p = np.ascontiguousarray(
        XT.reshape(KC, 128, S // 512, 512).transpose(1, 2, 0, 3)
        .reshape(128, -1))

    pos = np.asarray(position_ids)[0].astype(np.float64)
    inv = 1.0 / (ROPE_THETA ** (np.arange(0, HD, 2, dtype=np.float64) / HD))
    freqs = inv[:, None] * pos[None, :]  # [128, S]
    cosT = np.cos(freqs).astype(np.float32)
    sinT = np.sin(freqs).astype(np.float32)

    plan, maskb = _classify_mask(attention_mask, S)

    Wqkv = np.asarray(Wqkv, np.float32)
    Wo = np.asarray(Wo, np.float32)

    ident = np.eye(128, dtype=np.float32).astype(bf16)
    in_maps = []
    for c in range(N_CORES):
        g = c // (NH // NKV)
        wq = Wqkv[c * HD:(c + 1) * HD] * SCALE  # exact: SCALE = 2**-4
        wk = Wqkv[NH * HD + g * HD: NH * HD + (g + 1) * HD]
        wv = Wqkv[(NH + NKV) * HD + g * HD: (NH + NKV) * HD + (g + 1) * HD]
        # aux planes: even core of the pair computes pre-rope K, odd
        # computes hd-major V — the split lives entirely in the weights
        waux = wk if c % 2 == 0 else wv
        wqk = _pack_rows(
            np.concatenate([wq.T, waux.T], axis=1).astype(bf16))
        wot = _pack_rows(Wo[:, c * HD:(c + 1) * HD].T.astype(bf16))
        m = {
            "xt": xt_p, "wqk": wqk, "wo": wot,
            "cost": cosT, "sint": sinT, "ident": ident,
        }
        if maskb is not None:
            m["maskb"] = maskb
        in_maps.append(m)

    nc = _build(S, plan, 0 if maskb is None else maskb.shape[0])
    res = run_bass_kernel_spmd(nc, in_maps, list(range(N_CORES)),
                               trace=TRACE)
    acc = res.results[0]["out"].astype(np.float32)
    for c in range(1, N_CORES):
        acc += res.results[c]["out"].astype(np.float32)
    # out_p[p, t*HID + f] = out[t*128 + p, f]
    out = acc.reshape(128, S // 128, HID).transpose(1, 0, 2).reshape(S, HID)
    kernel.last_exec_time_ns = res.exec_time_ns
    kernel.last_results = res
    return np.ascontiguousarray(out)[None].astype(np.float32)


kernel.last_exec_time_ns = None
kernel.last_results = None
